# revision 27
# baseline (speedup 1.0000x reference)
"""Trainium2 Bass kernel for nn_BinClassDecoder (Bahdanau additive-attention
binary classifier decoder).

Contract: kernel(**inputs) takes the FULL unsharded inputs (numpy arrays, keys
as in reference.setup_inputs()) and returns the FULL [B, T, 1] float32 output.

Sharding: 8 NeuronCores; core c computes t-positions [8c, 8c+8) for ALL
batches (perfectly balanced in the dominant [B,t,s,d] tanh work even though
memory_lengths vary per batch).  The s-dimension is truncated per batch to
Lb = ceil2(len_b) -- everything past len_b is softmax-masked to zero, so the
truncation is exact.

v3 design (engine assignment driven by HW-measured rates):
  ACT (1.2 GHz/lane, ~420ns/instr): one big tanh per batch [128, 32, L],
    per-half softmax exp (fused row-sum via accum_out), final tanh;
    sigmoid via the tanh identity (Tanh/Exp/Copy share one act table);
    one PSUM->SBUF uh copy per large batch (load balance vs DVE).
  DVE (4x mode 0.26 ns/col for bf16 tensor_scalar; fp32/PSUM ops 1x): the
    256 strip broadcast-adds, remaining uh copies, reciprocals.
  PE  (bf16 1cy/row, fp32 4x slower -> everything bf16): uh projection,
    v-reduction via shifted-window stationary vectors, context matmul,
    all small projections.  The softmax mask initializes the two align
    PSUM banks via a rank-8 matmul; biases fold in as ones-row matmul
    contributions; softmax normalization folds into the A^T step by
    multiplying with diag(1/rowsum) in a regular matmul.
  Pipeline: batches orderd [mid, big..., small] across two 4-batch
    softmax halves; half 0's softmax/context runs under half 1's compute;
    uh projection runs 2 batches ahead.  GPSIMD cannot touch PSUM and is
    ~4 ns/col, so it gets nothing hot.  Long-open PSUM accumulation
    groups corrupt on HW -> wv is closed+copied early and re-added.
"""

import math
import os

import numpy as np

B, S, T = 8, 512, 64
ENC, WORD = 512, 512
NCORES = 8
TL = T // NCORES  # t-positions per core = 8
NEG = -1.0e30

BF16 = None  # filled lazily (ml_dtypes)


def _ceil32(x):
    return int(min(max(int(math.ceil(x / 32.0)) * 32, 32), 512))


def _ceil2(x):
    return int(min(max(int(math.ceil(x / 2.0)) * 2, 2), 512))


# ---------------------------------------------------------------------------
# v3 device kernel builder
# ---------------------------------------------------------------------------

def _build_nc_v3(Lb, reps=1):
    import concourse.bass as bass
    import concourse.tile as tile
    from concourse import bacc, mybir

    f32 = mybir.dt.float32
    bf16 = mybir.dt.bfloat16

    Lb = list(Lb)
    cum = [0]
    for b in range(B):
        cum.append(cum[-1] + Lb[b])
    SL = cum[-1]
    Sb = [(l + 127) // 128 for l in Lb]
    cumS = [0]
    for b in range(B):
        cumS.append(cumS[-1] + Sb[b])
    NS = cumS[-1]

    nc = bacc.Bacc()
    FP8 = os.environ.get("KERNEL_FP8", "1") == "1"
    FP8UH = FP8 and os.environ.get("KERNEL_FP8UH", "1") == "1"
    f8 = mybir.dt.float8e4

    d_mbT = nc.dram_tensor("mbT", [4, 128, SL], bf16, kind="ExternalInput")
    d_mbN = nc.dram_tensor("mbN", [NS, 128, ENC], bf16, kind="ExternalInput")
    d_wcT = nc.dram_tensor("wcT", [4, 128, ENC], bf16, kind="ExternalInput")
    d_wqT = nc.dram_tensor("wqT", [4, 128, ENC], bf16, kind="ExternalInput")
    d_wcwT = nc.dram_tensor("wcwT", [4, 128, WORD], bf16, kind="ExternalInput")
    d_wecT = nc.dram_tensor("wecT", [4, 128, WORD], bf16, kind="ExternalInput")
    d_weoT = nc.dram_tensor("weoT", [8, 128, WORD], bf16, kind="ExternalInput")
    d_tg = nc.dram_tensor("tg", [4, 128, 64], bf16, kind="ExternalInput")
    d_eh = nc.dram_tensor("eh", [8, 128, 64], bf16, kind="ExternalInput")
    d_vsh = nc.dram_tensor("vsh", [4, 128, 63], bf16, kind="ExternalInput")
    if FP8:
        d_vsh8 = nc.dram_tensor("vsh8", [2, 128, 32, 2, 32], f8,
                                kind="ExternalInput")
    if FP8UH:
        d_mbT8 = nc.dram_tensor("mbT8", [4, 128, SL], f8, kind="ExternalInput")
        d_wcT8 = nc.dram_tensor("wcT8", [2, 4, 128, 2, 128], f8,
                                kind="ExternalInput")
    d_vr = nc.dram_tensor("vr", [128, 4], bf16, kind="ExternalInput")
    d_bqr = nc.dram_tensor("bqr", [1, 512], bf16, kind="ExternalInput")
    d_bwr = nc.dram_tensor("bwr", [1, 512], bf16, kind="ExternalInput")
    d_hbv = nc.dram_tensor("hbv", [1, 1], f32, kind="ExternalInput")
    d_mskw = nc.dram_tensor("mskw", [8, 64], bf16, kind="ExternalInput")
    d_mskx = nc.dram_tensor("mskx", [8, 512], bf16, kind="ExternalInput")
    d_id = nc.dram_tensor("id64", [64, 64], f32, kind="ExternalInput")
    d_out = nc.dram_tensor("scores", [1, 64], f32, kind="ExternalOutput")
    DEBUG = os.environ.get("KERNEL_DEBUG") == "1"
    if DEBUG:
        d_dbg_al = nc.dram_tensor("dbg_al", [64, 512], f32, kind="ExternalOutput")
        d_dbg_A = nc.dram_tensor("dbg_A", [64, 512], f32, kind="ExternalOutput")
        d_dbg_wqb = nc.dram_tensor("dbg_wqb", [128, 4, 64], f32, kind="ExternalOutput")
        d_dbg_ct = nc.dram_tensor("dbg_ct", [128, 4, 64], f32, kind="ExternalOutput")
        d_dbg_ov = nc.dram_tensor("dbg_ov", [128, 4, 64], f32, kind="ExternalOutput")
        d_dbg_sc = nc.dram_tensor("dbg_sc", [1, 64], f32, kind="ExternalOutput")
        d_dbg_uh = nc.dram_tensor("dbg_uh", [128, 4, Lb[1]], f32,
                                  kind="ExternalOutput")
        d_dbg_pv = nc.dram_tensor("dbg_pv", [128, 4, 64], f32,
                                  kind="ExternalOutput")

    Tanh = mybir.ActivationFunctionType.Tanh
    Exp = mybir.ActivationFunctionType.Exp
    Copy = mybir.ActivationFunctionType.Copy
    ALU = mybir.AluOpType

    with tile.TileContext(nc) as tc:
        with (
            tc.tile_pool(name="consts", bufs=1) as consts,
            tc.tile_pool(name="work", bufs=1) as work,
            tc.tile_pool(name="strips", bufs=2) as strips,
            tc.tile_pool(name="strips8", bufs=2) as strips8,
            tc.tile_pool(name="ps_uh", bufs=3, space="PSUM") as ps_uh_pool,
            tc.tile_pool(name="ps_misc", bufs=1, space="PSUM") as ps_misc,
        ):
            sb_tg = consts.tile([128, 4, 64], bf16)
            nc.sync.dma_start(out=sb_tg, in_=d_tg.rearrange("a p j -> p a j"))
            sb_wqT = consts.tile([128, 4, ENC], bf16)
            nc.sync.dma_start(out=sb_wqT, in_=d_wqT.rearrange("a p d -> p a d"))
            sb_mbT = consts.tile([128, 4, SL], bf16)
            for kc in range(4):
                nc.sync.dma_start(out=sb_mbT[:, kc, :], in_=d_mbT[kc])
            sb_wcT = consts.tile([128, 4, ENC], bf16)
            nc.sync.dma_start(out=sb_wcT, in_=d_wcT.rearrange("a p d -> p a d"))
            sb_vsh = consts.tile([128, 4, 63], bf16)
            nc.sync.dma_start(out=sb_vsh, in_=d_vsh.rearrange("a p c -> p a c"))
            if FP8:
                sb_vsh8 = consts.tile([128, 2, 32, 2, 32], f8)
                nc.sync.dma_start(
                    out=sb_vsh8,
                    in_=d_vsh8.rearrange("g p r k c -> p g r k c"))
            if FP8UH:
                sb_mbT8 = consts.tile([128, 4, SL], f8)
                for kc in range(4):
                    nc.sync.dma_start(out=sb_mbT8[:, kc, :], in_=d_mbT8[kc])
                sb_wcT8 = consts.tile([128, 2, 4, 2, 128], f8)
                nc.sync.dma_start(
                    out=sb_wcT8,
                    in_=d_wcT8.rearrange("q a p k c -> p q a k c"))
            sb_mbN = consts.tile([128, NS, ENC], bf16)
            for g in range(4):
                lo = (NS * g) // 4
                hi = (NS * (g + 1)) // 4
                if hi > lo:
                    nc.sync.dma_start(
                        out=sb_mbN[:, lo:hi, :],
                        in_=d_mbN[lo:hi].rearrange("a p d -> p a d"))
            sb_wcwT = consts.tile([128, 4, WORD], bf16)
            nc.sync.dma_start(out=sb_wcwT, in_=d_wcwT.rearrange("a p d -> p a d"))
            sb_wecT = consts.tile([128, 4, WORD], bf16)
            nc.sync.dma_start(out=sb_wecT, in_=d_wecT.rearrange("a p d -> p a d"))
            sb_weoT = consts.tile([128, 8, WORD], bf16)
            nc.sync.dma_start(out=sb_weoT, in_=d_weoT.rearrange("a p d -> p a d"))
            sb_eh = consts.tile([128, 8, 64], bf16)
            nc.sync.dma_start(out=sb_eh, in_=d_eh.rearrange("a p j -> p a j"))
            sb_vr = consts.tile([128, 4], bf16)
            nc.sync.dma_start(out=sb_vr, in_=d_vr[:, :])
            sb_bqr = consts.tile([1, 512], bf16)
            nc.sync.dma_start(out=sb_bqr, in_=d_bqr[:, :])
            sb_bwr = consts.tile([1, 512], bf16)
            nc.sync.dma_start(out=sb_bwr, in_=d_bwr[:, :])
            sb_hbv = consts.tile([1, 1], f32)
            nc.sync.dma_start(out=sb_hbv, in_=d_hbv[:, :])
            sb_mskw = consts.tile([8, 64], bf16)
            nc.sync.dma_start(out=sb_mskw, in_=d_mskw[:, :])
            sb_mskx = consts.tile([8, 512], bf16)
            nc.sync.dma_start(out=sb_mskx, in_=d_mskx[:, :])
            sb_id = consts.tile([64, 64], f32)
            nc.sync.dma_start(out=sb_id, in_=d_id[:, :])
            sb_ones = consts.tile([1, 64], bf16)
            nc.vector.memset(sb_ones, 1.0)

            for _rep in range(reps):
                # ---- two align PSUM banks (one per 4-batch half), each
                #      initialized with that half's softmax mask rows ----
                ps_al0 = ps_misc.tile([128, 512], f32, tag="ps_al0", name="ps_al0")
                ps_al1 = ps_misc.tile([128, 512], f32, tag="ps_al1", name="ps_al1")
                nc.tensor.matmul(
                    ps_al0[0:32, :], sb_mskw[:, 0:32], sb_mskx,
                    start=True, stop=False, skip_group_check=True,
                )
                nc.tensor.matmul(
                    ps_al1[0:32, :], sb_mskw[:, 32:64], sb_mskx,
                    start=True, stop=False, skip_group_check=True,
                )

                # ---- wq projection (+ bias as ones-row matmul) ----
                ps_wq = ps_misc.tile([128, 4, 64], f32, tag="psA", name="ps_wq")
                for dc in range(4):
                    for kc in range(4):
                        nc.tensor.matmul(
                            ps_wq[:, dc, :],
                            sb_wqT[:, kc, dc * 128:(dc + 1) * 128],
                            sb_tg[:, kc, :],
                            start=(kc == 0), stop=False,
                            skip_group_check=True,
                        )
                    nc.tensor.matmul(
                        ps_wq[:, dc, :],
                        sb_bqr[0:1, dc * 128:(dc + 1) * 128],
                        sb_ones[0:1, :],
                        start=False, stop=True, skip_group_check=True,
                    )
                wqb = work.tile([128, 4, 64], f32)
                nc.vector.tensor_copy(out=wqb, in_=ps_wq)

                def emit_wv():
                    ps_wv = ps_misc.tile([128, 4, 64], f32, tag="psD",
                                         name="ps_wv")
                    for wc in range(4):
                        for kc in range(4):
                            nc.tensor.matmul(
                                ps_wv[:, wc, :],
                                sb_wcwT[:, kc, wc * 128:(wc + 1) * 128],
                                sb_tg[:, kc, :],
                                start=(kc == 0), stop=False,
                                skip_group_check=True,
                            )
                        for kc in range(8):
                            nc.tensor.matmul(
                                ps_wv[:, wc, :],
                                sb_weoT[:, kc, wc * 128:(wc + 1) * 128],
                                sb_eh[:, kc, :],
                                start=False, stop=False,
                                skip_group_check=True,
                            )
                        nc.tensor.matmul(
                            ps_wv[:, wc, :],
                            sb_bwr[0:1, wc * 128:(wc + 1) * 128],
                            sb_ones[0:1, :],
                            start=False, stop=(wc == 3), skip_group_check=True,
                        )
                    wv = work.tile([128, 4, 64], f32)
                    nc.vector.tensor_copy(out=wv, in_=ps_wv)
                    return wv

                # ---- per-batch pipeline ----
                uh_tiles = {}
                uh_ps = {}

                def uh_groups(b):
                    L = Lb[b]
                    g = 4 if L <= 128 else (2 if L <= 256 else 1)
                    return [(d0, g) for d0 in range(0, 4, g)]

                def emit_uh_mm(b):
                    L = Lb[b]
                    uh_tiles[b] = work.tile([128, 4, L], bf16, tag=f"uh{b}",
                                            name=f"uh{b}")
                    uh_ps[b] = []
                    for d0, g in uh_groups(b):
                        ps = ps_uh_pool.tile([128, g, L], f32, tag="ps_uh",
                                             name="ps_uh")
                        uh_ps[b].append(ps)
                        for i in range(g):
                            dc = d0 + i
                            if FP8UH:
                                for q in range(2):
                                    nc.tensor.matmul(
                                        ps[:, i, :],
                                        sb_wcT8[:, q, dc, :, :],
                                        sb_mbT8[:, 2 * q:2 * q + 2,
                                                cum[b]:cum[b] + L],
                                        start=(q == 0), stop=(q == 1),
                                        perf_mode=(
                                            mybir.MatmulPerfMode.DoubleRow),
                                    )
                            else:
                                for kc in range(4):
                                    nc.tensor.matmul(
                                        ps[:, i, :],
                                        sb_wcT[:, kc, dc * 128:(dc + 1) * 128],
                                        sb_mbT[:, kc, cum[b]:cum[b] + L],
                                        start=(kc == 0), stop=(kc == 3),
                                    )

                def emit_uh_copy(b):
                    # GPSIMD cannot access PSUM (BIR verifier); split the
                    # PSUM->SBUF copies between DVE and ACT (Copy func) to
                    # balance engine load.
                    groups = uh_groups(b)
                    for idx, (d0, g) in enumerate(groups):
                        if len(groups) == 4 and idx == 3:
                            nc.scalar.activation(
                                out=uh_tiles[b][:, d0:d0 + g, :],
                                in_=uh_ps[b][idx], func=Copy)
                        else:
                            nc.vector.tensor_copy(
                                out=uh_tiles[b][:, d0:d0 + g, :],
                                in_=uh_ps[b][idx],
                            )
                    del uh_ps[b]

                strips_tiles = {}

                def emit_adds(b):
                    L = Lb[b]
                    strip = strips.tile([128, 4 * TL, L], bf16, tag="strip",
                                        name="strip")
                    strips_tiles[b] = strip
                    for tl in range(TL):
                        j = b * TL + tl
                        for dc in range(4):
                            nc.vector.tensor_scalar_add(
                                out=strip[:, tl * 4 + dc, :],
                                in0=uh_tiles[b][:, dc, :],
                                scalar1=wqb[:, dc, j:j + 1],
                            )

                def emit_tanh_align(b):
                    L = Lb[b]
                    strip = strips_tiles.pop(b)
                    al = ps_al0 if b < 4 else ps_al1
                    if FP8:
                        # tanh writes fp8 strips; v-reduction runs DoubleRow
                        # (two dc-planes contracted per matmul at 0.5 cyc/row)
                        s8 = strips8.tile([128, 4 * TL, L], f8, tag="s8",
                                          name="s8")
                        nc.scalar.activation(out=s8, in_=strip, func=Tanh)
                        for tl in range(TL):
                            j = b * TL + tl
                            r = j % 32
                            for g in range(2):
                                last = ((b % 4 == 3) and (tl == TL - 1)
                                        and (g == 1))
                                nc.tensor.matmul(
                                    al[0:32, 0:L],
                                    sb_vsh8[:, g, r, :, :],
                                    s8[:, tl * 4 + 2 * g:tl * 4 + 2 * g + 2, :],
                                    start=False, stop=last,
                                    skip_group_check=True,
                                    perf_mode=mybir.MatmulPerfMode.DoubleRow,
                                )
                        return
                    nc.scalar.activation(out=strip, in_=strip, func=Tanh)
                    for tl in range(TL):
                        j = b * TL + tl
                        r = j % 32
                        for dc in range(4):
                            last = (b % 4 == 3) and (tl == TL - 1) and (dc == 3)
                            nc.tensor.matmul(
                                al[0:32, 0:L],
                                sb_vsh[:, dc, 31 - r:63 - r],
                                strip[:, tl * 4 + dc, :],
                                start=False, stop=last,
                                skip_group_check=True,
                            )

                A_half = [work.tile([32, 512], f32, tag="A0", name="A0"),
                          work.tile([32, 512], f32, tag="A1", name="A1")]
                AT = work.tile([128, 4, 64], bf16)
                ps_ct = ps_misc.tile([128, 4, 64], f32, tag="psD", name="ps_ct")

                def emit_softmax_half(h):
                    """exp+normalize+transpose+context for batches 4h..4h+3
                    (align rows live in ps_al{h}[0:32])."""
                    al = ps_al0 if h == 0 else ps_al1
                    r0, r1 = 32 * h, 32 * h + 32
                    Ah = A_half[h]
                    sums = work.tile([32, 1], f32, tag=f"sums{h}",
                                     name=f"sums{h}")
                    nc.scalar.activation(out=Ah, in_=al[0:32, :],
                                         func=Exp, accum_out=sums)
                    rec = work.tile([32, 1], f32, tag=f"rec{h}", name=f"rec{h}")
                    nc.vector.reciprocal(rec, sums)
                    # normalization folded into the transpose: A^T @ diag(1/sum)
                    diag = work.tile([32, 32], f32, tag=f"diag{h}",
                                     name=f"diag{h}")
                    nc.vector.tensor_scalar_mul(out=diag, in0=sb_id[0:32, 0:32],
                                                scalar1=rec)
                    ps_at = ps_misc.tile([128, 4, 32], f32, tag="psA",
                                         name="ps_at")
                    for sc in range(4):
                        nc.tensor.matmul(
                            ps_at[:, sc, :],
                            Ah[:, sc * 128:(sc + 1) * 128],
                            diag,
                            start=True, stop=True)
                    nc.vector.tensor_copy(out=AT[:, :, r0:r1], in_=ps_at)
                    for b in range(4 * h, 4 * h + 4):
                        for dc in range(4):
                            for sc in range(Sb[b]):
                                nc.tensor.matmul(
                                    ps_ct[:, dc, b * 8:b * 8 + 8],
                                    sb_mbN[:, cumS[b] + sc,
                                           dc * 128:(dc + 1) * 128],
                                    AT[:, sc, b * 8:b * 8 + 8],
                                    start=(sc == 0), stop=(sc == Sb[b] - 1),
                                    skip_group_check=True,
                                )

                emit_uh_mm(0)
                emit_uh_copy(0)
                emit_uh_mm(1)
                emit_uh_copy(1)
                wv = None
                for b in range(B):
                    if b + 2 < B:
                        emit_uh_mm(b + 2)
                    emit_adds(b)
                    if b == 0:
                        wv = emit_wv()
                    if b + 2 < B:
                        emit_uh_copy(b + 2)
                    emit_tanh_align(b)
                    if b == 3:
                        emit_softmax_half(0)
                emit_softmax_half(1)
                cT = work.tile([128, 4, 64], bf16)
                nc.vector.tensor_copy(out=cT, in_=ps_ct)

                # ---- cont projection + wv add, then tanh ----
                ps_ov = ps_misc.tile([128, 4, 64], f32, tag="psC", name="ps_ov")
                for wc in range(4):
                    for kc in range(4):
                        nc.tensor.matmul(
                            ps_ov[:, wc, :],
                            sb_wecT[:, kc, wc * 128:(wc + 1) * 128],
                            cT[:, kc, :],
                            start=(kc == 0), stop=(kc == 3),
                            skip_group_check=True,
                        )
                nc.vector.tensor_add(out=ps_ov, in0=ps_ov, in1=wv)
                ov = work.tile([128, 4, 64], bf16)
                nc.scalar.activation(out=ov, in_=ps_ov, func=Tanh)

                # ---- score + sigmoid (via tanh identity) ----
                ps_sc = ps_misc.tile([128, 64], f32, tag="psA", name="ps_sc")
                for wc in range(4):
                    nc.tensor.matmul(
                        ps_sc[0:1, :],
                        sb_vr[:, wc:wc + 1],
                        ov[:, wc, :],
                        start=(wc == 0), stop=(wc == 3),
                    )
                esb = work.tile([1, 64], f32)
                nc.scalar.activation(out=esb, in_=ps_sc[0:1, :], func=Tanh,
                                     bias=sb_hbv[0:1, :], scale=0.5)
                osb = work.tile([1, 64], f32)
                nc.vector.tensor_scalar(
                    out=osb, in0=esb, scalar1=0.5, scalar2=0.5,
                    op0=ALU.mult, op1=ALU.add,
                )
                nc.sync.dma_start(out=d_out[:, :], in_=osb)

                if DEBUG:
                    alsb = work.tile([64, 512], f32, name="dbg_alsb")
                    nc.vector.tensor_copy(out=alsb[0:32, :], in_=ps_al0[0:32, :])
                    nc.vector.tensor_copy(out=alsb[32:64, :], in_=ps_al1[0:32, :])
                    nc.sync.dma_start(out=d_dbg_al[:, :], in_=alsb)
                    nc.sync.dma_start(out=d_dbg_A[0:32, :], in_=A_half[0])
                    nc.sync.dma_start(out=d_dbg_A[32:64, :], in_=A_half[1])
                    nc.sync.dma_start(out=d_dbg_wqb[:, :, :], in_=wqb)
                    ctf = work.tile([128, 4, 64], f32, name="dbg_ctf")
                    nc.vector.tensor_copy(out=ctf, in_=cT)
                    nc.sync.dma_start(out=d_dbg_ct[:, :, :], in_=ctf)
                    ovf = work.tile([128, 4, 64], f32, name="dbg_ovf")
                    nc.vector.tensor_copy(out=ovf, in_=ov)
                    nc.sync.dma_start(out=d_dbg_ov[:, :, :], in_=ovf)
                    scf = work.tile([1, 64], f32, name="dbg_scf")
                    nc.vector.tensor_copy(out=scf, in_=ps_sc[0:1, :])
                    nc.sync.dma_start(out=d_dbg_sc[:, :], in_=scf)
                    uhf = work.tile([128, 4, Lb[1]], f32, name="dbg_uhf")
                    nc.vector.tensor_copy(out=uhf, in_=uh_tiles[1])
                    nc.sync.dma_start(out=d_dbg_uh[:, :, :], in_=uhf)
                    pvf = work.tile([128, 4, 64], f32, name="dbg_pvf")
                    nc.vector.tensor_copy(out=pvf, in_=ps_ov)
                    nc.sync.dma_start(out=d_dbg_pv[:, :, :], in_=pvf)

    nc.compile()
    return nc


def _build_nc_v1(Lb, reps=1):
    """v1 structure: per-(batch,t) strips, single align bank, softmax at end.
    Previous baseline (123991 ns); kept for A/B."""
    import concourse.bass as bass
    import concourse.tile as tile
    from concourse import bacc, mybir

    f32 = mybir.dt.float32
    bf16 = mybir.dt.bfloat16

    Lb = list(Lb)
    cum = [0]
    for b in range(B):
        cum.append(cum[-1] + Lb[b])
    SL = cum[-1]
    Sb = [(l + 127) // 128 for l in Lb]
    cumS = [0]
    for b in range(B):
        cumS.append(cumS[-1] + Sb[b])
    NS = cumS[-1]

    nc = bacc.Bacc()
    FP8 = os.environ.get("KERNEL_FP8", "1") == "1"
    FP8UH = FP8 and os.environ.get("KERNEL_FP8UH", "1") == "1"
    f8 = mybir.dt.float8e4

    d_mbT = nc.dram_tensor("mbT", [4, 128, SL], bf16, kind="ExternalInput")
    d_mbN = nc.dram_tensor("mbN", [NS, 128, ENC], f32, kind="ExternalInput")
    d_wcT = nc.dram_tensor("wcT", [4, 128, ENC], bf16, kind="ExternalInput")
    d_wqT = nc.dram_tensor("wqT", [4, 128, ENC], f32, kind="ExternalInput")
    d_wcwT = nc.dram_tensor("wcwT", [4, 128, WORD], f32, kind="ExternalInput")
    d_wecT = nc.dram_tensor("wecT", [4, 128, WORD], f32, kind="ExternalInput")
    d_weoT = nc.dram_tensor("weoT", [8, 128, WORD], f32, kind="ExternalInput")
    d_tg = nc.dram_tensor("tg", [4, 128, 64], f32, kind="ExternalInput")
    d_eh = nc.dram_tensor("eh", [8, 128, 64], f32, kind="ExternalInput")
    d_vsh = nc.dram_tensor("vsh", [4, 128, 63], bf16, kind="ExternalInput")
    if FP8:
        d_vsh8 = nc.dram_tensor("vsh8", [2, 128, 32, 2, 32], f8,
                                kind="ExternalInput")
    if FP8UH:
        d_mbT8 = nc.dram_tensor("mbT8", [4, 128, SL], f8, kind="ExternalInput")
        d_wcT8 = nc.dram_tensor("wcT8", [2, 4, 128, 2, 128], f8,
                                kind="ExternalInput")
    d_vr = nc.dram_tensor("vr", [128, 4], f32, kind="ExternalInput")
    d_bq = nc.dram_tensor("bq", [128, 4], f32, kind="ExternalInput")
    d_bw = nc.dram_tensor("bw", [128, 4], f32, kind="ExternalInput")
    d_nbv = nc.dram_tensor("nbv", [1, 1], f32, kind="ExternalInput")
    d_msk = nc.dram_tensor("msk", [64, 512], f32, kind="ExternalInput")
    d_id = nc.dram_tensor("id64", [64, 64], f32, kind="ExternalInput")
    d_out = nc.dram_tensor("scores", [1, 64], f32, kind="ExternalOutput")

    Tanh = mybir.ActivationFunctionType.Tanh
    Exp = mybir.ActivationFunctionType.Exp

    with tile.TileContext(nc) as tc:
        with (
            tc.tile_pool(name="consts", bufs=1) as consts,
            tc.tile_pool(name="work", bufs=1) as work,
            tc.tile_pool(name="strips", bufs=6) as strips,
            tc.tile_pool(name="ps_uh", bufs=2, space="PSUM") as ps_uh_pool,
            tc.tile_pool(name="ps_misc", bufs=1, space="PSUM") as ps_misc,
        ):
            sb_tg = consts.tile([128, 4, 64], f32)
            nc.sync.dma_start(out=sb_tg, in_=d_tg.rearrange("a p j -> p a j"))
            sb_wqT = consts.tile([128, 4, ENC], f32)
            nc.sync.dma_start(out=sb_wqT, in_=d_wqT.rearrange("a p d -> p a d"))
            sb_bq = consts.tile([128, 4], f32)
            nc.sync.dma_start(out=sb_bq, in_=d_bq[:, :])
            sb_mbT = consts.tile([128, 4, SL], bf16)
            for kc in range(4):
                nc.sync.dma_start(out=sb_mbT[:, kc, :], in_=d_mbT[kc])
            sb_wcT = consts.tile([128, 4, ENC], bf16)
            nc.sync.dma_start(out=sb_wcT, in_=d_wcT.rearrange("a p d -> p a d"))
            sb_vsh = consts.tile([128, 4, 63], bf16)
            nc.sync.dma_start(out=sb_vsh, in_=d_vsh.rearrange("a p c -> p a c"))
            if FP8:
                sb_vsh8 = consts.tile([128, 2, 32, 2, 32], f8)
                nc.sync.dma_start(
                    out=sb_vsh8,
                    in_=d_vsh8.rearrange("g p r k c -> p g r k c"))
            if FP8UH:
                sb_mbT8 = consts.tile([128, 4, SL], f8)
                for kc in range(4):
                    nc.sync.dma_start(out=sb_mbT8[:, kc, :], in_=d_mbT8[kc])
                sb_wcT8 = consts.tile([128, 2, 4, 2, 128], f8)
                nc.sync.dma_start(
                    out=sb_wcT8,
                    in_=d_wcT8.rearrange("q a p k c -> p q a k c"))
            sb_msk = consts.tile([64, 512], f32)
            nc.sync.dma_start(out=sb_msk, in_=d_msk[:, :])
            sb_id = consts.tile([64, 64], f32)
            nc.sync.dma_start(out=sb_id, in_=d_id[:, :])
            sb_mbN = consts.tile([128, NS, ENC], f32)
            for g in range(4):
                lo = (NS * g) // 4
                hi = (NS * (g + 1)) // 4
                if hi > lo:
                    nc.sync.dma_start(
                        out=sb_mbN[:, lo:hi, :],
                        in_=d_mbN[lo:hi].rearrange("a p d -> p a d"))
            sb_wcwT = consts.tile([128, 4, WORD], f32)
            nc.sync.dma_start(out=sb_wcwT, in_=d_wcwT.rearrange("a p d -> p a d"))
            sb_wecT = consts.tile([128, 4, WORD], f32)
            nc.sync.dma_start(out=sb_wecT, in_=d_wecT.rearrange("a p d -> p a d"))
            sb_weoT = consts.tile([128, 8, WORD], f32)
            nc.sync.dma_start(out=sb_weoT, in_=d_weoT.rearrange("a p d -> p a d"))
            sb_eh = consts.tile([128, 8, 64], f32)
            nc.sync.dma_start(out=sb_eh, in_=d_eh.rearrange("a p j -> p a j"))
            sb_vr = consts.tile([128, 4], f32)
            nc.sync.dma_start(out=sb_vr, in_=d_vr[:, :])
            sb_bw = consts.tile([128, 4], f32)
            nc.sync.dma_start(out=sb_bw, in_=d_bw[:, :])
            sb_nbv = consts.tile([1, 1], f32)
            nc.sync.dma_start(out=sb_nbv, in_=d_nbv[:, :])

            sb_zero = consts.tile([1, 576], f32)
            nc.vector.memset(sb_zero, 0.0)

            for _rep in range(reps):
                ps_wq = ps_misc.tile([128, 4, 64], f32, tag="psA", name="ps_wq")
                for dc in range(4):
                    for kc in range(4):
                        nc.tensor.matmul(
                            ps_wq[:, dc, :],
                            sb_wqT[:, kc, dc * 128:(dc + 1) * 128],
                            sb_tg[:, kc, :],
                            start=(kc == 0), stop=(kc == 3),
                        )
                wqb = work.tile([128, 4, 64], f32)
                for dc in range(4):
                    nc.vector.tensor_scalar_add(
                        out=wqb[:, dc, :], in0=ps_wq[:, dc, :],
                        scalar1=sb_bq[:, dc:dc + 1],
                    )

                ps_al = ps_misc.tile([128, 512], f32, tag="ps_al", name="ps_al")
                nc.tensor.matmul(
                    ps_al[0:64, :],
                    sb_zero[0:1, 0:64],
                    sb_zero[0:1, 0:512],
                    start=True, stop=False, skip_group_check=True,
                )

                ps_wv = ps_misc.tile([128, 4, 64], f32, tag="psB", name="ps_wv")
                for wc in range(4):
                    for kc in range(4):
                        nc.tensor.matmul(
                            ps_wv[:, wc, :],
                            sb_wcwT[:, kc, wc * 128:(wc + 1) * 128],
                            sb_tg[:, kc, :],
                            start=(kc == 0), stop=False,
                            skip_group_check=True,
                        )
                    for kc in range(8):
                        nc.tensor.matmul(
                            ps_wv[:, wc, :],
                            sb_weoT[:, kc, wc * 128:(wc + 1) * 128],
                            sb_eh[:, kc, :],
                            start=False, stop=(kc == 7),
                            skip_group_check=True,
                        )
                wv = work.tile([128, 4, 64], f32)
                nc.vector.tensor_copy(out=wv[:, :, :], in_=ps_wv[:, :, :])

                for b in range(B):
                    L = Lb[b]
                    uh_b = work.tile([128, 4, L], bf16, tag=f"uh{b}", name=f"uh{b}")
                    for dc in range(4):
                        ps = ps_uh_pool.tile([128, 512], f32, tag="ps_uh",
                                             name="ps_uh")
                        for kc in range(4):
                            nc.tensor.matmul(
                                ps[:, 0:L],
                                sb_wcT[:, kc, dc * 128:(dc + 1) * 128],
                                sb_mbT[:, kc, cum[b]:cum[b] + L],
                                start=(kc == 0), stop=(kc == 3),
                            )
                        nc.vector.tensor_copy(out=uh_b[:, dc, :], in_=ps[:, 0:L])

                    TP = 2  # t-positions fused per tanh instruction
                    for t0 in range(0, TL, TP):
                        strip = strips.tile([128, TP * 4, L], bf16, tag="strip",
                                            name="strip")
                        for ti in range(TP):
                            j = b * TL + t0 + ti
                            for dc in range(4):
                                nc.vector.tensor_scalar_add(
                                    out=strip[:, ti * 4 + dc, :],
                                    in0=uh_b[:, dc, :],
                                    scalar1=wqb[:, dc, j:j + 1],
                                )
                        nc.scalar.activation(out=strip[:, :, :],
                                             in_=strip[:, :, :], func=Tanh)
                        for ti in range(TP):
                            j = b * TL + t0 + ti
                            pos = j % 32
                            blk = j // 32
                            last = (b == B - 1) and (t0 + ti == TL - 1)
                            for dc in range(4):
                                nc.tensor.matmul(
                                    ps_al[32 * blk:32 * blk + 32, 0:L],
                                    sb_vsh[:, dc, 31 - pos:63 - pos],
                                    strip[:, ti * 4 + dc, :],
                                    start=False,
                                    stop=(last and dc == 3),
                                    skip_group_check=True,
                                )

                nc.vector.tensor_add(out=ps_al[0:64, :], in0=ps_al[0:64, :],
                                     in1=sb_msk)
                A = work.tile([64, 512], f32)
                sums = work.tile([64, 1], f32)
                nc.scalar.activation(out=A, in_=ps_al[0:64, :], func=Exp,
                                     accum_out=sums)
                rec = work.tile([64, 1], f32)
                nc.vector.reciprocal(rec, sums)
                nc.vector.tensor_scalar_mul(out=A, in0=A, scalar1=rec)

                ps_at = ps_misc.tile([128, 4, 64], f32, tag="psA", name="ps_at")
                for sc in range(4):
                    nc.tensor.transpose(ps_at[:, sc, :],
                                        A[0:64, sc * 128:(sc + 1) * 128], sb_id)
                AT = work.tile([128, 4, 64], f32)
                nc.vector.tensor_copy(out=AT[:, :, :], in_=ps_at[:, :, :])

                ps_ct = ps_misc.tile([128, 4, 64], f32, tag="psB", name="ps_ct")
                for b in range(B):
                    for dc in range(4):
                        for sc in range(Sb[b]):
                            nc.tensor.matmul(
                                ps_ct[:, dc, b * 8:b * 8 + 8],
                                sb_mbN[:, cumS[b] + sc, dc * 128:(dc + 1) * 128],
                                AT[:, sc, b * 8:b * 8 + 8],
                                start=(sc == 0), stop=(sc == Sb[b] - 1),
                                skip_group_check=True,
                            )
                cT = work.tile([128, 4, 64], f32)
                nc.vector.tensor_copy(out=cT[:, :, :], in_=ps_ct[:, :, :])

                ps_ov = ps_misc.tile([128, 4, 64], f32, tag="psC", name="ps_ov")
                ov = work.tile([128, 4, 64], f32)
                for wc in range(4):
                    for kc in range(4):
                        nc.tensor.matmul(
                            ps_ov[:, wc, :],
                            sb_wecT[:, kc, wc * 128:(wc + 1) * 128],
                            cT[:, kc, :],
                            start=(kc == 0), stop=(kc == 3),
                            skip_group_check=True,
                        )
                    nc.vector.tensor_add(
                        out=ps_ov[:, wc, :], in0=ps_ov[:, wc, :], in1=wv[:, wc, :])
                    nc.scalar.activation(
                        out=ov[:, wc, :], in_=ps_ov[:, wc, :], func=Tanh,
                        bias=sb_bw[:, wc:wc + 1],
                    )

                ps_sc = ps_misc.tile([128, 64], f32, tag="psA", name="ps_sc")
                for wc in range(4):
                    nc.tensor.matmul(
                        ps_sc[0:1, :],
                        sb_vr[:, wc:wc + 1],
                        ov[:, wc, :],
                        start=(wc == 0), stop=(wc == 3),
                    )
                esb = work.tile([1, 64], f32)
                nc.scalar.activation(out=esb, in_=ps_sc[0:1, :], func=Exp,
                                     bias=sb_nbv[0:1, :], scale=-1.0)
                nc.vector.tensor_scalar_add(out=esb, in0=esb, scalar1=1.0)
                osb = work.tile([1, 64], f32)
                nc.vector.reciprocal(osb, esb)
                nc.sync.dma_start(out=d_out[:, :], in_=osb)

    nc.compile()
    return nc


# ---------------------------------------------------------------------------
# host-side input prep
# ---------------------------------------------------------------------------

def _kv():
    return os.environ.get("KERNEL_V", "3")


def _prep(inputs):
    global BF16
    import ml_dtypes
    BF16 = ml_dtypes.bfloat16
    v3 = _kv() == "3"

    enc_state = np.asarray(inputs["enc_state"], dtype=np.float32)
    mb = np.asarray(inputs["memory_bank"], dtype=np.float32)      # [S, B, ENC]
    tgt = np.asarray(inputs["tgt"], dtype=np.float32)             # [T, B, WORD]
    lens = np.asarray(inputs["memory_lengths"]).astype(np.int64)  # [B]
    Wq = np.asarray(inputs["Wq"], dtype=np.float32)
    bq = np.asarray(inputs["bq"], dtype=np.float32)
    Wc = np.asarray(inputs["Wc"], dtype=np.float32)
    v_w = np.asarray(inputs["v_w"], dtype=np.float32)
    W_enc_out = np.asarray(inputs["W_enc_out"], dtype=np.float32)
    b_enc_out = np.asarray(inputs["b_enc_out"], dtype=np.float32)
    W_enc_ctx = np.asarray(inputs["W_enc_ctx"], dtype=np.float32)
    b_enc_ctx = np.asarray(inputs["b_enc_ctx"], dtype=np.float32)
    W_cw = np.asarray(inputs["W_cw"], dtype=np.float32)
    b_cw = np.asarray(inputs["b_cw"], dtype=np.float32)
    w_vrank = np.asarray(inputs["w_vrank"], dtype=np.float32)
    b_vrank = np.asarray(inputs["b_vrank"], dtype=np.float32)

    # batch -> pipeline-slot order: modest batch first (quick pipeline fill),
    # largest in the first softmax half (its softmax hides under the second
    # half's compute), smallest last (short pipeline drain).
    rnd = _ceil2 if v3 else _ceil32
    Lb_raw = [rnd(int(l)) for l in lens]
    srt = [int(i) for i in np.argsort(np.asarray(Lb_raw, np.int64),
                                      kind="stable")]
    if v3:
        perm = tuple(srt[i] for i in (1, 5, 6, 7, 4, 3, 2, 0))
    else:
        perm = tuple(srt)
    mb = mb[:, perm, :]
    tgt = tgt[:, perm, :]
    lens = lens[list(perm)]
    enc_state = enc_state[:, perm, :]

    Lb = tuple(Lb_raw[p] for p in perm)
    cum = [0]
    for b in range(B):
        cum.append(cum[-1] + Lb[b])
    SL = cum[-1]
    Sb = [(l + 127) // 128 for l in Lb]
    cumS = [0]
    for b in range(B):
        cumS.append(cumS[-1] + Sb[b])
    NS = cumS[-1]

    nf = BF16 if v3 else np.float32

    mbT = np.zeros([4, 128, SL], dtype=BF16)
    mbN = np.zeros([NS, 128, ENC], dtype=nf)
    for b in range(B):
        seg = mb[:Lb[b], b, :]                       # [Lb, ENC]
        mbT[:, :, cum[b]:cum[b + 1]] = seg.T.reshape(4, 128, Lb[b]).astype(BF16)
        segN = mb[:Sb[b] * 128, b, :]
        mbN[cumS[b]:cumS[b + 1]] = segN.reshape(Sb[b], 128, ENC).astype(nf)

    wcT = np.ascontiguousarray(Wc.T.reshape(4, 128, ENC)).astype(BF16)
    wqT = np.ascontiguousarray(Wq.T.reshape(4, 128, ENC)).astype(nf)
    wcwT = np.ascontiguousarray(W_cw.T.reshape(4, 128, WORD)).astype(nf)
    wecT = np.ascontiguousarray(W_enc_ctx.T.reshape(4, 128, WORD)).astype(nf)
    weoT = np.ascontiguousarray(W_enc_out.T.reshape(8, 128, WORD)).astype(nf)

    enc_hidden = np.concatenate([enc_state[0], enc_state[1]], axis=-1)  # [B,1024]
    ehT = enc_hidden.T                                                  # [1024,B]
    eh = np.ascontiguousarray(np.repeat(ehT, TL, axis=1).reshape(8, 128, 64)
                              ).astype(nf)

    vsh = np.zeros([4, 128, 63], dtype=BF16)
    for dc in range(4):
        vsh[dc, :, 31] = v_w[dc * 128:(dc + 1) * 128].astype(BF16)
    if v3:
        from concourse import mybir as _mb
        F8NP = _mb.dt.np(_mb.dt.float8e4)
        # dense per-output-row weight windows: w[g, p, r, kt, i] is
        # v_{2g+kt}[p] on the diagonal i == r, else 0
        vsh8 = np.zeros([2, 128, 32, 2, 32], dtype=F8NP)
        for g in range(2):
            for kt in range(2):
                dc = 2 * g + kt
                vcol = v_w[dc * 128:(dc + 1) * 128].astype(F8NP)
                for r in range(32):
                    vsh8[g, :, r, kt, r] = vcol

    id64 = np.eye(64, dtype=np.float32)

    common = {
        "mbT": mbT, "mbN": mbN, "wcT": wcT, "wqT": wqT, "wcwT": wcwT,
        "wecT": wecT, "weoT": weoT, "eh": eh, "vsh": vsh, "id64": id64,
    }
    if v3:
        common["vsh8"] = vsh8
        common["mbT8"] = mbT.astype(F8NP)
        wcT8 = np.zeros([2, 4, 128, 2, 128], dtype=F8NP)
        wct_full = Wc.T.astype(np.float32)   # [ENC(in), ENC(out)]
        for q in range(2):
            for dc in range(4):
                for kt in range(2):
                    kc = 2 * q + kt
                    wcT8[q, dc, :, kt, :] = wct_full[
                        kc * 128:(kc + 1) * 128,
                        dc * 128:(dc + 1) * 128].astype(F8NP)
        common["wcT8"] = wcT8
        common["vr"] = np.ascontiguousarray(w_vrank.reshape(4, 128).T
                                            ).astype(BF16)
        common["bqr"] = bq.reshape(1, 512).astype(BF16)
        common["bwr"] = (b_enc_out + b_enc_ctx + b_cw).reshape(1, 512
                                                              ).astype(BF16)
        common["hbv"] = np.array([[0.5 * float(b_vrank)]], dtype=np.float32)
        mskw = np.zeros([8, 64], dtype=BF16)
        for b in range(B):
            mskw[b, b * TL:(b + 1) * TL] = 1.0
        common["mskw"] = mskw
        mskx = np.zeros([8, 512], dtype=np.float32)
        for b in range(B):
            mskx[b, int(min(max(lens[b], 0), 512)):] = NEG
        common["mskx"] = mskx.astype(BF16)
    else:
        common["vr"] = np.ascontiguousarray(w_vrank.reshape(4, 128).T)
        common["bq"] = np.ascontiguousarray(bq.reshape(4, 128).T)
        common["bw"] = np.ascontiguousarray(
            (b_enc_out + b_enc_ctx + b_cw).reshape(4, 128).T)
        common["nbv"] = np.array([[-float(b_vrank)]], dtype=np.float32)
        msk = np.zeros([64, 512], dtype=np.float32)
        for b in range(B):
            msk[b * TL:(b + 1) * TL, int(min(max(lens[b], 0), 512)):] = NEG
        common["msk"] = msk

    in_maps = []
    for c in range(NCORES):
        # tg[kc, p, j] with j = pos*8 + tl for t_global = 8c + tl
        x = tgt[c * TL:(c + 1) * TL]                 # [TL, B(perm), WORD]
        x2 = np.ascontiguousarray(x.transpose(2, 1, 0).reshape(4, 128, 64)
                                  ).astype(nf)
        m = dict(common)
        m["tg"] = x2
        in_maps.append(m)
    return Lb, in_maps, perm


_NC_CACHE = {}


def _get_nc(Lb, reps=1):
    v = _kv()
    key = (tuple(Lb), reps, v, os.environ.get("KERNEL_FP8", "1"),
           os.environ.get("KERNEL_FP8UH", "1"))
    nc = _NC_CACHE.get(key)
    if nc is None:
        if v == "1":
            nc = _build_nc_v1(Lb, reps=reps)
        else:
            nc = _build_nc_v3(Lb, reps=reps)
        _NC_CACHE[key] = nc
    return nc


def _assemble(results, perm):
    full = np.zeros([B, T, 1], dtype=np.float32)
    for c in range(NCORES):
        out = np.asarray(results[c]["scores"]).reshape(64)
        for pos in range(B):
            full[perm[pos], c * TL:(c + 1) * TL, 0] = out[pos * TL:(pos + 1) * TL]
    return full


def kernel(**inputs):
    from concourse.bass_utils import run_bass_kernel_spmd

    Lb, in_maps, perm = _prep(inputs)
    nc = _get_nc(Lb)
    res = run_bass_kernel_spmd(nc, in_maps, core_ids=list(range(NCORES)))
    return _assemble(res.results, perm)


# -- helper for test.py: build a reusable jitted runner (timing loops) -------

def make_runner(reps=1, **inputs):
    """Returns (run_once, time_reps, call_timed). The shard_map'ed executable
    is built ONCE (one neuronx compile); repeat calls measure steady-state
    dispatch+execute time with inputs already resident on-device.  With
    reps>1 the NEFF contains the whole compute body repeated `reps` times
    (for launch-overhead-free HW timing via deltas)."""
    import jax
    import numpy as np
    from jax.experimental.shard_map import shard_map
    from jax.sharding import Mesh, NamedSharding, PartitionSpec
    from concourse import bass2jax, mybir
    from concourse.bass2jax import (
        _bass_exec_p, install_neuronx_cc_hook, partition_id_tensor,
    )

    install_neuronx_cc_hook()
    Lb, in_maps, perm = _prep(inputs)
    nc = _get_nc(Lb, reps=reps)
    pid_name = nc.partition_id_tensor.name if nc.partition_id_tensor else None

    in_names, out_names, out_avals, zero_outs = [], [], [], []
    for alloc in nc.m.functions[0].allocations:
        import concourse.mybir as mybir_
        if not isinstance(alloc, mybir_.MemoryLocationSet):
            continue
        name = alloc.memorylocations[0].name
        if alloc.kind == "ExternalInput":
            if name != pid_name:
                in_names.append(name)
        elif alloc.kind == "ExternalOutput":
            shape = tuple(alloc.tensor_shape)
            dtype = mybir_.dt.np(alloc.dtype)
            out_names.append(name)
            out_avals.append(jax.core.ShapedArray(shape, dtype))
            zero_outs.append(np.zeros(shape, dtype))
    n_params = len(in_names)
    n_outs = len(out_avals)
    all_in_names = list(in_names) + list(out_names)
    if pid_name is not None:
        all_in_names.append(pid_name)
    donate = tuple(range(n_params, n_params + n_outs))

    def _body(*args):
        operands = list(args)
        if pid_name is not None:
            operands.append(partition_id_tensor())
        outs = _bass_exec_p.bind(
            *operands,
            out_avals=tuple(out_avals),
            in_names=tuple(all_in_names),
            out_names=tuple(out_names),
            lowering_input_output_aliases=(),
            sim_require_finite=True,
            sim_require_nnan=True,
            nc=nc,
        )
        return tuple(outs)

    devices = jax.devices()[:NCORES]
    mesh = Mesh(np.asarray(devices), ("core",))
    in_specs = (PartitionSpec("core"),) * (n_params + n_outs)
    out_specs = (PartitionSpec("core"),) * n_outs
    sharded = jax.jit(
        shard_map(_body, mesh=mesh, in_specs=in_specs, out_specs=out_specs,
                  check_rep=False),
        donate_argnums=donate, keep_unused=True,
    )
    concat_in = [
        np.concatenate([np.asarray(in_maps[c][name]) for c in range(NCORES)],
                       axis=0)
        for name in in_names
    ]
    shard = NamedSharding(mesh, PartitionSpec("core"))
    concat_in_dev = [jax.device_put(a, shard) for a in concat_in]
    zshapes = [(NCORES * z.shape[0], *z.shape[1:]) for z in zero_outs]
    zdtypes = [z.dtype for z in zero_outs]

    def _zeros_dev():
        return [jax.device_put(np.zeros(s, d), shard)
                for s, d in zip(zshapes, zdtypes)]

    def run_once():
        outs = sharded(*concat_in_dev, *_zeros_dev())
        res = [
            {name: np.asarray(outs[i]).reshape(NCORES, *out_avals[i].shape)[c]
             for i, name in enumerate(out_names)}
            for c in range(NCORES)
        ]
        return _assemble(res, perm)

    def time_reps(reps=50):
        import time
        outs = sharded(*concat_in_dev, *_zeros_dev())   # warm
        jax.block_until_ready(outs)
        zs = [_zeros_dev() for _ in range(reps)]
        t0 = time.perf_counter()
        all_outs = []
        for r in range(reps):
            all_outs.append(sharded(*concat_in_dev, *zs[r]))
        jax.block_until_ready(all_outs)
        dt = (time.perf_counter() - t0) / reps
        return dt

    def call_timed():
        import time
        z = _zeros_dev()
        t0 = time.perf_counter()
        outs = sharded(*concat_in_dev, *z)
        jax.block_until_ready(outs)
        return time.perf_counter() - t0

    return run_once, time_reps, call_timed
